# revision 2
# baseline (speedup 1.0000x reference)
"""FlowNetC correlation (kernel_size=1, max_disp=20, stride2=2) on 8 Trainium2 cores.

Problem: inputs input1, input2 of shape [8, 256, 64, 96] fp32; output
[8, 441, 64, 96] fp32 with
  out[b, i*21+j, y, x] = (1/256) * sum_c in1[b,c,y,x] * in2[b,c,y+2i-20,x+2j-20]
(zero where the in2 index is out of range).

Sharding: data-parallel over batch - core b handles batch element b.

Per-core strategy: tile (y, x) into 48 parity-separated blocks of 8x16 = 128
pixels.  For each block the TensorEngine computes the banded product
P[m, (r,u)] = sum_c in1[c, y_m, x_m] * in2[c, r, u] over the block's
displacement window, accumulating fp32 in PSUM.

Mixed-precision inputs (vs. the fp16 baseline) cut input DMA from 6.29 MB to
4.72 MB per core against the serialized ~360 B/ns DMA pipe, pulling total DMA
(with int8 stores) to ~24 us — under the ~25.8 us TensorEngine stream — so
the kernel is PE-bound end to end:
  * channels 0-127 as fp8e3m4 (one K=128 matmul per chunk; e3m4 keeps
    1 row/cycle like fp16 at half the bytes),
  * channels 128-255 as fp8e4m3 hi+lo pairs consumed by two DoubleRow
    matmuls per chunk: [ah,al]x[bh,bl] + [ah,al]x[bl,bh] (the second uses a
    reversed k-tile AP, no data duplication) = (ah+al)*(bh+bl), i.e. the
    full 4-term product, so the e4m3 half contributes ~no error.  DoubleRow
    packs K=256 rows into one instruction at 0.5 cycles/row, so the two DR
    matmuls cost the same PE time as one fp16 matmul.
Measured end-to-end rel err ~1.65e-2 (e3m4 half ~1.34e-2, int8 out ~0.95e-2),
under the 2e-2 gate.

PSUM bands drain to SBUF as *int8* with a fixed scale (127/64 covers ~4
sigma of the dot-product distribution).  Each (block, half-band) gets one
PSUM bank (ring of 8); DoubleRow's moving-AP limit (2*n*nu <= 512) splits a
half-band into <=2 accumulation sub-chunks, but the drain stays one op per
half-band (ACT / DVE alternating) so drain-engine time is unchanged.  The 4
same-geometry blocks of each (y0, x0) quad pack into one [128, 4*ntot] int8
staging tile so every store keeps >=1872 B contiguous runs (full DMA rate).

Loads are sliced in consumption order (g0 row windows and block slices ahead
of the 2x-larger g1 ones) so the first matmul fires ~2.5 us in; dummy
matmuls on zeroed SBUF keep the PE busy from ~0.4 us so the p-state ramp
(full clock only after 3 us continuously busy) is paid during the lead-in.
The host scatters the valid banded entries into the final output (fixed
sparse permutation) and undoes the int8 scale.
"""

import numpy as np

C, H, W = 256, 64, 96
D = 21
PADV = 20
B = 8
N_CORES = 8
BY, BX = 8, 16
NBLK = 48
QSTRIDE = 512          # psum bank size in fp32 elements
ST_FREE = 4096         # int8 staging/out free size per quad (4 * max ntot = 3744)
SCALE = 127.0 / 64.0   # int8 quantization scale (exact in fp32)
SUBMAX = 256           # max columns per DoubleRow matmul (moving free 2*256 <= 512)

# load schedule: (kind 1=in1-blocks / 2=in2-rows, grp, begin, end)
# grp 0 = e3m4 tensors (channels 0-127), grp 1 = e4m3 hi/lo (channels 128-255)
LOADS = [
    (2, 0, 0, 18), (1, 0, 0, 4), (1, 1, 0, 4), (2, 1, 0, 18),
    (1, 0, 4, 12), (1, 1, 4, 12), (2, 0, 18, 36), (2, 1, 18, 36),  # gy0
    (1, 0, 12, 24), (1, 1, 12, 24), (2, 0, 36, 52), (2, 1, 36, 52),  # gy1
    (2, 0, 52, 64), (2, 1, 52, 64), (1, 0, 24, 36), (1, 1, 24, 36),  # gy2
    (1, 0, 36, 48), (1, 1, 36, 48),  # gy3
]
N_WARMUP = 44          # fine (128-row) dummy matmuls warming the PE p-state


def _block_geometry():
    blocks = []
    for y0 in (0, 16, 32, 48):
        # large (x0=32, nu=36) quads first within each y0-group
        for x0 in (32, 0, 64):
            for py in (0, 1):
                for px in (0, 1):
                    ys = [y0 + py + 2 * b for b in range(BY)]
                    xs = [x0 + px + 2 * a for a in range(BX)]
                    r_lo = ys[0] - PADV
                    while r_lo < 0:
                        r_lo += 2
                    r_hi = min(ys[-1] + PADV, H - 1)
                    rs = list(range(r_lo, r_hi + 1, 2))
                    u_lo = xs[0] - PADV
                    while u_lo < 0:
                        u_lo += 2
                    u_hi = min(xs[-1] + PADV, W - 1)
                    us = list(range(u_lo, u_hi + 1, 2))
                    nu = len(us)
                    nr = len(rs)
                    # split rows in half: two pipelined half-bands per block,
                    # each draining as one op from its own PSUM bank
                    n0 = nr // 2
                    assert n0 * nu <= QSTRIDE
                    chunks = [(0, n0), (n0, nr - n0)]
                    blocks.append(dict(ys=ys, xs=xs, rs=rs, us=us, chunks=chunks))
    assert len(blocks) == NBLK
    return blocks


_BLOCKS = _block_geometry()
_GATHER = None
_PROGRAM = None

# per-block pixel coordinates: YM[blk, m], XM[blk, m] with m = b*BX + a
_YM = np.array([np.repeat(g["ys"], BX) for g in _BLOCKS])
_XM = np.array([np.tile(g["xs"], BY) for g in _BLOCKS])


def _quad_ntot(qi):
    g = _BLOCKS[4 * qi]
    return len(g["rs"]) * len(g["us"])


def _build_gather():
    """Flat indices such that O.flat[dst] = R.flat[src] for one core."""
    dst_list, src_list = [], []
    for blk, g in enumerate(_BLOCKS):
        ys = np.asarray(g["ys"])
        xs = np.asarray(g["xs"])
        rs = np.asarray(g["rs"])
        us = np.asarray(g["us"])
        nu = len(us)
        ntot = len(rs) * nu
        y_m = np.repeat(ys, BX)
        x_m = np.tile(xs, BY)
        nr = len(rs)
        m_idx = np.arange(128)[:, None, None]
        ir = np.arange(nr)[None, :, None]
        iu = np.arange(nu)[None, None, :]
        i = (rs[None, :, None] - y_m[:, None, None] + PADV) // 2
        j = (us[None, None, :] - x_m[:, None, None] + PADV) // 2
        valid = (i >= 0) & (i < D) & (j >= 0) & (j < D)
        d = i * D + j
        dst = (d * H + y_m[:, None, None]) * W + x_m[:, None, None]
        src = ((blk // 4) * 128 + m_idx) * ST_FREE + (blk % 4) * ntot + ir * nu + iu
        bcast = np.broadcast_arrays(dst, src, valid)
        dst_list.append(bcast[0][valid])
        src_list.append(bcast[1][valid])
    return np.concatenate(dst_list), np.concatenate(src_list)


def _gather_indices():
    global _GATHER
    if _GATHER is None:
        _GATHER = _build_gather()
    return _GATHER


def _build_program():
    from contextlib import ExitStack

    import concourse.bacc as bacc
    import concourse.mybir as mybir
    import concourse.tile as tile

    g0_dt = mybir.dt.float8e3
    g1_dt = mybir.dt.float8e4
    out_dt = mybir.dt.int8

    nc = bacc.Bacc("TRN2", target_bir_lowering=False, debug=False)
    # in1 pre-packed on the host (pixel blocks):
    #   in1g0[p, blk, m]    = e3m4(in1[p,      YM[blk,m], XM[blk,m]])
    #   in1g1[p, t, blk, m] = e4m3 hi/lo t of in1[128+p, YM[blk,m], XM[blk,m]]
    in1g0_d = nc.dram_tensor("in1g0", [128, NBLK, 128], g0_dt, kind="ExternalInput")
    in1g1_d = nc.dram_tensor("in1g1", [128, 2, NBLK, 128], g1_dt, kind="ExternalInput")
    in2g0_d = nc.dram_tensor("in2g0", [128, H, W], g0_dt, kind="ExternalInput")
    in2g1_d = nc.dram_tensor("in2g1", [128, 2, H, W], g1_dt, kind="ExternalInput")
    out_d = nc.dram_tensor(
        "out", [NBLK // 4, 128, ST_FREE], out_dt, kind="ExternalOutput"
    )

    with ExitStack() as ctx:
        tc = ctx.enter_context(tile.TileContext(nc))
        inp_pool = ctx.enter_context(tc.tile_pool(name="inp", bufs=1))
        psum_pool = ctx.enter_context(tc.tile_pool(name="psum", bufs=8, space="PSUM"))
        out_pool = ctx.enter_context(tc.tile_pool(name="outp", bufs=12))

        in1g0_s = inp_pool.tile([128, NBLK, 128], g0_dt)
        in1g1_s = inp_pool.tile([128, 2, NBLK, 128], g1_dt)
        in2g0_s = inp_pool.tile([128, H, W], g0_dt)
        in2g1_s = inp_pool.tile([128, 2, H, W], g1_dt)
        wz = inp_pool.tile([128, 128], g0_dt)

        # Fine-grained input loads on the sync (SP) HWDGE path, emitted in
        # consumption order so the DMA pipe feeds the PE just-in-time and the
        # first matmul fires as early as possible.
        def l1(grp, b0, b1):
            if grp == 0:
                nc.sync.dma_start(in1g0_s[:, b0:b1, :], in1g0_d[:, b0:b1, :])
            else:
                nc.sync.dma_start(in1g1_s[:, :, b0:b1, :], in1g1_d[:, :, b0:b1, :])

        def l2(grp, r0, r1):
            if grp == 0:
                nc.sync.dma_start(in2g0_s[:, r0:r1, :], in2g0_d[:, r0:r1, :])
            else:
                nc.sync.dma_start(in2g1_s[:, :, r0:r1, :], in2g1_d[:, :, r0:r1, :])

        for kind, grp, a0, a1 in LOADS:
            (l1 if kind == 1 else l2)(grp, a0, a1)

        # PE p-state warmup source: zeroed fp8 tile (Pool engine: free
        # earliest, so dummies start ~0.4 us)
        nc.gpsimd.memset(wz[:, :], 0.0)

        # --- half-band software pipeline -------------------------------
        # Each (block, half) is a PSUM bank-granular unit.  Matmuls per
        # accumulation sub-chunk (<=256 columns, DoubleRow moving limit):
        #   1. e3m4 K=128  (channels 0-127),            start=True
        #   2. e4m3 DoubleRow K=256 [ah,al]x[bh,bl]
        #   3. e4m3 DoubleRow K=256 [ah,al]x[bl,bh],    stop=True
        # then one scaled int8 drain per half-band, alternating ACT / DVE.
        # Chunk order per y0-group: all first-halves then all second-halves
        # (gy3 per-block for a short store tail), so every matmul's in2 row
        # window is resident when the pipeline reaches it.
        chunk_list = []
        for gy in range(4):
            if gy < 3:
                for ci in (0, 1):
                    for blk in range(12 * gy, 12 * gy + 12):
                        chunk_list.append((blk, ci))
            else:
                for blk in range(36, 48):
                    for ci in (0, 1):
                        chunk_list.append((blk, ci))

        warm = psum_pool.tile([128, QSTRIDE], mybir.dt.float32, tag="bk", name="warm")
        for _ in range(N_WARMUP):
            nc.tensor.matmul(
                warm[:, :128], wz[:, :128], wz[:, :128], start=True, stop=True
            )

        st_tiles = {}
        drained = {qi: 0 for qi in range(NBLK // 4)}
        n_drains = 0

        def emit_chunk(blk, ci):
            nonlocal n_drains
            g = _BLOCKS[blk]
            off, n = g["chunks"][ci]
            nu = len(g["us"])
            u0 = g["us"][0]
            ntot = len(g["rs"]) * nu
            bk = psum_pool.tile(
                [128, QSTRIDE], mybir.dt.float32, tag="bk", name=f"bk{blk}_{ci}"
            )
            # row split so every sub-chunk has <= SUBMAX columns
            nhi = SUBMAX // nu
            subs = []
            o = 0
            while o < n:
                k = min(nhi, n - o)
                subs.append((o, k))
                o += k
            for o, k in subs:
                r0 = g["rs"][off + o]
                dst = bk[:, o * nu : (o + k) * nu]
                rsl = slice(r0, r0 + 2 * k - 1, 2)
                usl = slice(u0, u0 + 2 * nu - 1, 2)
                nc.tensor.matmul(
                    dst,
                    in1g0_s[:, blk, :],
                    in2g0_s[:, rsl, usl],
                    start=True,
                    stop=False,
                )
                nc.tensor.matmul(
                    dst,
                    in1g1_s[:, :, blk, :],
                    in2g1_s[:, :, rsl, usl],
                    start=False,
                    stop=False,
                    perf_mode=mybir.MatmulPerfMode.DoubleRow,
                )
                nc.tensor.matmul(
                    dst,
                    in1g1_s[:, :, blk, :],
                    in2g1_s[:, ::-1, rsl, usl],
                    start=False,
                    stop=True,
                    perf_mode=mybir.MatmulPerfMode.DoubleRow,
                )
            qi = blk // 4
            if qi not in st_tiles:
                st_tiles[qi] = out_pool.tile(
                    [128, ST_FREE], out_dt, tag="st", name=f"st{qi}"
                )
            st = st_tiles[qi]
            base = (blk % 4) * ntot
            dst = st[:, base + off * nu : base + (off + n) * nu]
            if n_drains % 2 == 0:
                nc.scalar.mul(dst, bk[:, : n * nu], SCALE)
            else:
                nc.vector.tensor_scalar_mul(dst, bk[:, : n * nu], SCALE)
            n_drains += 1
            drained[qi] += 1
            width = 4 * ntot
            if qi == NBLK // 4 - 1:
                # final quad (per-block chunk order): store in block pairs so
                # the first half streams while the last blocks finish
                if drained[qi] == 4:
                    nc.sync.dma_start(
                        out_d[qi, :, : width // 2], st[:, : width // 2]
                    )
                elif drained[qi] == 8:
                    nc.sync.dma_start(
                        out_d[qi, :, width // 2 : width], st[:, width // 2 : width]
                    )
            elif drained[qi] == 8:
                nc.sync.dma_start(out_d[qi, :, :width], st[:, :width])

        for blk, ci in chunk_list:
            emit_chunk(blk, ci)

    nc.compile()
    return nc


def _program():
    global _PROGRAM
    if _PROGRAM is None:
        _PROGRAM = _build_program()
    return _PROGRAM


def _np_dtypes():
    import ml_dtypes

    return ml_dtypes.float8_e3m4, ml_dtypes.float8_e4m3


def _prep_in1(x):
    """[256, 64, 96] -> (in1g0 [128,NBLK,128] e3m4, in1g1 [128,2,NBLK,128] e4m3)."""
    e3, e4 = _np_dtypes()
    g = x[:, _YM, _XM]  # [256, NBLK, 128]
    g0 = np.ascontiguousarray(g[:128].astype(e3))
    hi = g[128:].astype(e4)
    lo = (g[128:] - hi.astype(np.float32)).astype(e4)
    return g0, np.ascontiguousarray(np.stack([hi, lo], axis=1))


def _prep_in2(x):
    """[256, 64, 96] -> (in2g0 [128,H,W] e3m4, in2g1 [128,2,H,W] e4m3)."""
    e3, e4 = _np_dtypes()
    g0 = np.ascontiguousarray(x[:128].astype(e3))
    hi = x[128:].astype(e4)
    lo = (x[128:] - hi.astype(np.float32)).astype(e4)
    return g0, np.ascontiguousarray(np.stack([hi, lo], axis=1))


def make_in_maps(input1, input2):
    in1 = np.asarray(input1, dtype=np.float32)
    in2 = np.asarray(input2, dtype=np.float32)
    maps = []
    for b in range(B):
        a0, a1 = _prep_in1(in1[b])
        b0, b1 = _prep_in2(in2[b])
        maps.append({"in1g0": a0, "in1g1": a1, "in2g0": b0, "in2g1": b1})
    return maps


def extract_output(R):
    """R: [NBLK//4, 128, ST_FREE] int8 device result -> [441, 64, 96] fp32."""
    dst, src = _gather_indices()
    O = np.zeros(D * D * H * W, dtype=np.float32)
    O[dst] = R.reshape(-1)[src].astype(np.float32)
    O *= np.float32(1.0 / (SCALE * C))
    return O.reshape(D * D, H, W)


def run_spmd(in_maps, **kwargs):
    from concourse import bass_utils

    return bass_utils.run_bass_kernel_spmd(
        _program(), in_maps, core_ids=list(range(N_CORES)), **kwargs
    )


def kernel(input1, input2):
    in_maps = make_in_maps(input1, input2)
    res = run_spmd(in_maps)
    return np.stack([extract_output(res.results[b]["out"]) for b in range(B)])


# revision 16
# speedup vs baseline: 1.2402x; 1.2402x over previous
"""FlowNetC correlation (kernel_size=1, max_disp=20, stride2=2) on 8 Trainium2 cores.

Problem: inputs input1, input2 of shape [8, 256, 64, 96] fp32; output
[8, 441, 64, 96] fp32 with
  out[b, i*21+j, y, x] = (1/256) * sum_c in1[b,c,y,x] * in2[b,c,y+2i-20,x+2j-20]
(zero where the in2 index is out of range).

Sharding: data-parallel over batch - core b handles batch element b.

Per-core strategy: tile (y, x) into 48 parity-separated blocks of 8x16 = 128
pixels.  For each block the TensorEngine computes the banded product
P[m, (r,u)] = sum_c in1[c, y_m, x_m] * in2[c, r, u] over the block's
displacement window, accumulating fp32 in PSUM.

Mixed-precision inputs (vs. the fp16 baseline) cut input DMA from 6.29 MB to
4.72 MB per core against the serialized ~360 B/ns DMA pipe, pulling total DMA
(with int8 stores) to ~24 us — under the ~25.8 us TensorEngine stream — so
the kernel is PE-bound end to end:
  * channels 0-127 as fp8e3m4 (one K=128 matmul per chunk; e3m4 keeps
    1 row/cycle like fp16 at half the bytes),
  * channels 128-255 as fp8e4m3 hi+lo pairs consumed by two DoubleRow
    matmuls per chunk: [ah,al]x[bh,bl] + [ah,al]x[bl,bh] (the second uses a
    reversed k-tile AP, no data duplication) = (ah+al)*(bh+bl), i.e. the
    full 4-term product, so the e4m3 half contributes ~no error.  DoubleRow
    packs K=256 rows into one instruction at 0.5 cycles/row, so the two DR
    matmuls cost the same PE time as one fp16 matmul.
Measured end-to-end rel err ~1.65e-2 (e3m4 half ~1.34e-2, int8 out ~0.95e-2),
under the 2e-2 gate.

PSUM bands drain to SBUF as *int8* with a fixed scale (127/64 covers ~4
sigma of the dot-product distribution).  Each (block, half-band) gets one
PSUM bank (ring of 8); DoubleRow's moving-AP limit (2*n*nu <= 512) splits a
half-band into <=2 accumulation sub-chunks, but the drain stays one op per
half-band (ACT / DVE alternating) so drain-engine time is unchanged.  The 4
same-geometry blocks of each (y0, x0) quad pack into one [128, 4*ntot] int8
staging tile so every store keeps >=1872 B contiguous runs (full DMA rate).

Loads are sliced in consumption order (g0 row windows and block slices ahead
of the 2x-larger g1 ones) so the first matmul fires ~2.5 us in; dummy
matmuls on zeroed SBUF keep the PE busy from ~0.4 us so the p-state ramp
(full clock only after 3 us continuously busy) is paid during the lead-in.
The host scatters the valid banded entries into the final output (fixed
sparse permutation) and undoes the int8 scale.
"""

import numpy as np

C, H, W = 256, 64, 96
D = 21
PADV = 20
B = 8
N_CORES = 8
BY, BX = 8, 16
NBLK = 48
QSTRIDE = 512          # psum bank size in fp32 elements
ST_FREE = 4096         # int8 staging/out free size per quad (4 * max ntot = 3744)
SCALE = 127.0 / 64.0   # int8 quantization scale (exact in fp32)
SUBMAX = 256           # max columns per DoubleRow matmul (moving free 2*256 <= 512)

# load schedule: (kind 1=in1-blocks / 2=in2-rows, grp, begin, end)
# grp 0 = e3m4 tensors (channels 0-127), grp 1 = e4m3 hi/lo (channels 128-255)
# g1 slices lead: the DoubleRow stream (emitted ahead of the e3m4/drain
# stream by LAG chunks) is what gates the pipeline start.
LOADS = [
    (1, 1, 0, 4), (2, 1, 0, 18), (1, 1, 4, 12), (2, 0, 0, 18), (1, 0, 0, 12),
    (2, 1, 18, 36), (2, 0, 18, 36), (1, 1, 12, 24), (1, 0, 12, 24),  # gy0
    (2, 1, 36, 52), (2, 0, 36, 52), (1, 1, 24, 36), (1, 0, 24, 36),  # gy1
    (2, 1, 52, 64), (2, 0, 52, 64), (1, 1, 36, 48), (1, 0, 36, 48),  # gy2/3
]
import os as _os

N_WARMUP = int(_os.environ.get("KW", "30"))  # dummy matmuls warming the PE p-state
N_FILLER = int(_os.environ.get("KF", "0"))   # chunks given an interleaved dummy
LAG = int(_os.environ.get("KL", "0"))        # DR-stream lead over e3m4+drain stream


def _block_geometry():
    blocks = []
    for y0 in (0, 16, 32, 48):
        # large (x0=32, nu=36) quads first within each y0-group
        for x0 in (32, 0, 64):
            for py in (0, 1):
                for px in (0, 1):
                    ys = [y0 + py + 2 * b for b in range(BY)]
                    xs = [x0 + px + 2 * a for a in range(BX)]
                    r_lo = ys[0] - PADV
                    while r_lo < 0:
                        r_lo += 2
                    r_hi = min(ys[-1] + PADV, H - 1)
                    rs = list(range(r_lo, r_hi + 1, 2))
                    u_lo = xs[0] - PADV
                    while u_lo < 0:
                        u_lo += 2
                    u_hi = min(xs[-1] + PADV, W - 1)
                    us = list(range(u_lo, u_hi + 1, 2))
                    nu = len(us)
                    nr = len(rs)
                    # split rows in half: two pipelined half-bands per block,
                    # each one PSUM bank (<= 512 cols) draining as one op
                    n0 = nr // 2
                    assert n0 * nu <= QSTRIDE and (nr - n0) * nu <= QSTRIDE
                    chunks = [(0, n0), (n0, nr - n0)]
                    blocks.append(dict(ys=ys, xs=xs, rs=rs, us=us, chunks=chunks))
    assert len(blocks) == NBLK
    return blocks


_BLOCKS = _block_geometry()
_GATHER = None
_PROGRAM = None

# per-block pixel coordinates: YM[blk, m], XM[blk, m] with m = b*BX + a
_YM = np.array([np.repeat(g["ys"], BX) for g in _BLOCKS])
_XM = np.array([np.tile(g["xs"], BY) for g in _BLOCKS])


def _quad_ntot(qi):
    g = _BLOCKS[4 * qi]
    return len(g["rs"]) * len(g["us"])


def _build_gather():
    """Flat indices such that O.flat[dst] = R.flat[src] for one core."""
    dst_list, src_list = [], []
    for blk, g in enumerate(_BLOCKS):
        ys = np.asarray(g["ys"])
        xs = np.asarray(g["xs"])
        rs = np.asarray(g["rs"])
        us = np.asarray(g["us"])
        nu = len(us)
        ntot = len(rs) * nu
        y_m = np.repeat(ys, BX)
        x_m = np.tile(xs, BY)
        nr = len(rs)
        m_idx = np.arange(128)[:, None, None]
        ir = np.arange(nr)[None, :, None]
        iu = np.arange(nu)[None, None, :]
        i = (rs[None, :, None] - y_m[:, None, None] + PADV) // 2
        j = (us[None, None, :] - x_m[:, None, None] + PADV) // 2
        valid = (i >= 0) & (i < D) & (j >= 0) & (j < D)
        d = i * D + j
        dst = (d * H + y_m[:, None, None]) * W + x_m[:, None, None]
        src = ((blk // 4) * 128 + m_idx) * ST_FREE + (blk % 4) * ntot + ir * nu + iu
        bcast = np.broadcast_arrays(dst, src, valid)
        dst_list.append(bcast[0][valid])
        src_list.append(bcast[1][valid])
    return np.concatenate(dst_list), np.concatenate(src_list)


def _gather_indices():
    global _GATHER
    if _GATHER is None:
        _GATHER = _build_gather()
    return _GATHER


def _build_program():
    from contextlib import ExitStack

    import concourse.bacc as bacc
    import concourse.mybir as mybir
    import concourse.tile as tile

    g0_dt = mybir.dt.float8e3
    g1_dt = mybir.dt.float8e4
    out_dt = mybir.dt.int8

    nc = bacc.Bacc("TRN2", target_bir_lowering=False, debug=False)
    # in1 pre-packed on the host (pixel blocks):
    #   in1g0[p, blk, m]    = e3m4(in1[p,      YM[blk,m], XM[blk,m]])
    #   in1g1[p, t, blk, m] = e4m3 hi/lo t of in1[128+p, YM[blk,m], XM[blk,m]]
    in1g0_d = nc.dram_tensor("in1g0", [128, NBLK, 128], g0_dt, kind="ExternalInput")
    in1g1_d = nc.dram_tensor("in1g1", [128, NBLK, 2, 128], g1_dt, kind="ExternalInput")
    in2g0_d = nc.dram_tensor("in2g0", [128, H, W], g0_dt, kind="ExternalInput")
    in2g1_d = nc.dram_tensor("in2g1", [128, H, 2, W], g1_dt, kind="ExternalInput")
    out_d = nc.dram_tensor(
        "out", [NBLK // 4, 128, ST_FREE], out_dt, kind="ExternalOutput"
    )

    with ExitStack() as ctx:
        tc = ctx.enter_context(tile.TileContext(nc))
        inp_pool = ctx.enter_context(tc.tile_pool(name="inp", bufs=1))
        psum_pool = ctx.enter_context(tc.tile_pool(name="psum", bufs=8, space="PSUM"))
        out_pool = ctx.enter_context(tc.tile_pool(name="outp", bufs=12))

        in1g0_s = inp_pool.tile([128, NBLK, 128], g0_dt)
        in1g1_s = inp_pool.tile([128, NBLK, 2, 128], g1_dt)
        in2g0_s = inp_pool.tile([128, H, W], g0_dt)
        in2g1_s = inp_pool.tile([128, H, 2, W], g1_dt)
        wz = inp_pool.tile([128, 128], g0_dt)

        # Fine-grained input loads on the sync (SP) HWDGE path, emitted in
        # consumption order so the DMA pipe feeds the PE just-in-time and the
        # first matmul fires as early as possible.
        def l1(grp, b0, b1):
            if grp == 0:
                nc.sync.dma_start(in1g0_s[:, b0:b1, :], in1g0_d[:, b0:b1, :])
            else:
                nc.sync.dma_start(in1g1_s[:, b0:b1, :, :], in1g1_d[:, b0:b1, :, :])

        def l2(grp, r0, r1):
            if grp == 0:
                nc.sync.dma_start(in2g0_s[:, r0:r1, :], in2g0_d[:, r0:r1, :])
            else:
                nc.sync.dma_start(in2g1_s[:, r0:r1, :, :], in2g1_d[:, r0:r1, :, :])

        for kind, grp, a0, a1 in LOADS:
            (l1 if kind == 1 else l2)(grp, a0, a1)

        # PE p-state warmup source: zeroed fp8 tile (Pool engine: free
        # earliest, so dummies start ~0.4 us)
        nc.gpsimd.memset(wz[:, :], 0.0)

        # --- half-band software pipeline -------------------------------
        # Each (block, half) is a PSUM bank-granular unit.  Matmuls per
        # accumulation sub-chunk (<=256 columns, DoubleRow moving limit):
        #   1. e3m4 K=128  (channels 0-127),            start=True
        #   2. e4m3 DoubleRow K=256 [ah,al]x[bh,bl]
        #   3. e4m3 DoubleRow K=256 [ah,al]x[bl,bh],    stop=True
        # then one scaled int8 drain per half-band, alternating ACT / DVE.
        # Chunk order per y0-group: all first-halves then all second-halves
        # (gy3 per-block for a short store tail), so every matmul's in2 row
        # window is resident when the pipeline reaches it.
        chunk_list = []
        for gy in range(4):
            if gy < 3:
                for ci in (0, 1):
                    for blk in range(12 * gy, 12 * gy + 12):
                        chunk_list.append((blk, ci))
            else:
                for blk in range(36, 48):
                    for ci in (0, 1):
                        chunk_list.append((blk, ci))

        warm = psum_pool.tile([128, QSTRIDE], mybir.dt.float32, tag="bk", name="warm")
        for _ in range(N_WARMUP):
            nc.tensor.matmul(
                warm[:, :128], wz[:, :128], wz[:, :128], start=True, stop=True
            )

        bank_tiles = {}
        st_tiles = {}
        drained = {qi: 0 for qi in range(NBLK // 4)}
        n_drains = 0

        def chunk_geom(blk, ci):
            g = _BLOCKS[blk]
            off, n = g["chunks"][ci]
            nu = len(g["us"])
            u0 = g["us"][0]
            r0 = g["rs"][off]
            rsl = slice(r0, r0 + 2 * n - 1, 2)
            usl = slice(u0, u0 + 2 * nu - 1, 2)
            return g, off, n, nu, rsl, usl

        def emit_dr(blk, ci):
            """DoubleRow pair (channels 128-255) — opens the PSUM bank."""
            g, off, n, nu, rsl, usl = chunk_geom(blk, ci)
            bk = psum_pool.tile(
                [128, QSTRIDE], mybir.dt.float32, tag="bk", name=f"bk{blk}_{ci}"
            )
            bank_tiles[(blk, ci)] = bk
            dst = bk[:, : n * nu]
            nc.tensor.matmul(
                dst,
                in1g1_s[:, blk, :, :],
                in2g1_s[:, rsl, :, usl].transpose([0, 2, 1, 3]),
                start=True,
                stop=False,
                perf_mode=mybir.MatmulPerfMode.DoubleRow,
            )
            nc.tensor.matmul(
                dst,
                in1g1_s[:, blk, :, :],
                in2g1_s[:, rsl, ::-1, usl].transpose([0, 2, 1, 3]),
                start=False,
                stop=False,
                perf_mode=mybir.MatmulPerfMode.DoubleRow,
            )

        def emit_m1_drain(blk, ci):
            """e3m4 matmul (channels 0-127), then the int8 drain + stores."""
            nonlocal n_drains
            g, off, n, nu, rsl, usl = chunk_geom(blk, ci)
            ntot = len(g["rs"]) * nu
            bk = bank_tiles.pop((blk, ci))
            nc.tensor.matmul(
                bk[:, : n * nu],
                in1g0_s[:, blk, :],
                in2g0_s[:, rsl, usl],
                start=False,
                stop=True,
            )
            qi = blk // 4
            if qi not in st_tiles:
                st_tiles[qi] = out_pool.tile(
                    [128, ST_FREE], out_dt, tag="st", name=f"st{qi}"
                )
            st = st_tiles[qi]
            base = (blk % 4) * ntot
            dst = st[:, base + off * nu : base + (off + n) * nu]
            if n_drains % 2 == 0:
                nc.scalar.mul(dst, bk[:, : n * nu], SCALE)
            else:
                nc.vector.tensor_scalar_mul(dst, bk[:, : n * nu], SCALE)
            n_drains += 1
            drained[qi] += 1
            width = 4 * ntot
            if qi == NBLK // 4 - 1:
                # final quad (per-block chunk order): first half early on the
                # SP queue; the last half via the DVE queue so its issue isn't
                # serialized behind SP's store backlog at the very end
                if drained[qi] == 4:
                    nc.sync.dma_start(
                        out_d[qi, :, : width // 2], st[:, : width // 2]
                    )
                elif drained[qi] == 8:
                    nc.scalar.dma_start(
                        out_d[qi, :, width // 2 : width], st[:, width // 2 : width]
                    )
            elif drained[qi] == 8:
                nc.sync.dma_start(out_d[qi, :, :width], st[:, :width])

        pending = []
        for j, (blk, ci) in enumerate(chunk_list):
            emit_dr(blk, ci)
            if j < N_FILLER:
                nc.tensor.matmul(
                    warm[:, :128], wz[:, :128], wz[:, :128], start=True, stop=True
                )
            pending.append((blk, ci))
            while len(pending) > LAG:
                emit_m1_drain(*pending.pop(0))
        while pending:
            emit_m1_drain(*pending.pop(0))

    nc.compile()
    return nc


def _program():
    global _PROGRAM
    if _PROGRAM is None:
        _PROGRAM = _build_program()
    return _PROGRAM


def _np_dtypes():
    import ml_dtypes

    return ml_dtypes.float8_e3m4, ml_dtypes.float8_e4m3


def _prep_in1(x):
    """[256, 64, 96] -> (in1g0 [128,NBLK,128] e3m4, in1g1 [128,NBLK,2,128] e4m3)."""
    e3, e4 = _np_dtypes()
    g = x[:, _YM, _XM]  # [256, NBLK, 128]
    g0 = np.ascontiguousarray(g[:128].astype(e3))
    hi = g[128:].astype(e4)
    lo = (g[128:] - hi.astype(np.float32)).astype(e4)
    return g0, np.ascontiguousarray(np.stack([hi, lo], axis=2))


def _prep_in2(x):
    """[256, 64, 96] -> (in2g0 [128,H,W] e3m4, in2g1 [128,H,2,W] e4m3)."""
    e3, e4 = _np_dtypes()
    g0 = np.ascontiguousarray(x[:128].astype(e3))
    hi = x[128:].astype(e4)
    lo = (x[128:] - hi.astype(np.float32)).astype(e4)
    return g0, np.ascontiguousarray(np.stack([hi, lo], axis=2))


def make_in_maps(input1, input2):
    in1 = np.asarray(input1, dtype=np.float32)
    in2 = np.asarray(input2, dtype=np.float32)
    maps = []
    for b in range(B):
        a0, a1 = _prep_in1(in1[b])
        b0, b1 = _prep_in2(in2[b])
        maps.append({"in1g0": a0, "in1g1": a1, "in2g0": b0, "in2g1": b1})
    return maps


def extract_output(R):
    """R: [NBLK//4, 128, ST_FREE] int8 device result -> [441, 64, 96] fp32."""
    dst, src = _gather_indices()
    O = np.zeros(D * D * H * W, dtype=np.float32)
    O[dst] = R.reshape(-1)[src].astype(np.float32)
    O *= np.float32(1.0 / (SCALE * C))
    return O.reshape(D * D, H, W)


def run_spmd(in_maps, **kwargs):
    from concourse import bass_utils

    return bass_utils.run_bass_kernel_spmd(
        _program(), in_maps, core_ids=list(range(N_CORES)), **kwargs
    )


def kernel(input1, input2):
    in_maps = make_in_maps(input1, input2)
    res = run_spmd(in_maps)
    return np.stack([extract_output(res.results[b]["out"]) for b in range(B)])


# revision 40
# speedup vs baseline: 1.3566x; 1.0939x over previous
"""FlowNetC correlation (kernel_size=1, max_disp=20, stride2=2) on 8 Trainium2 cores.

Problem: inputs input1, input2 of shape [8, 256, 64, 96] fp32; output
[8, 441, 64, 96] fp32 with
  out[b, i*21+j, y, x] = (1/256) * sum_c in1[b,c,y,x] * in2[b,c,y+2i-20,x+2j-20]
(zero where the in2 index is out of range).

Sharding: data-parallel over batch - core b handles batch element b.

Per-core strategy: tile (y, x) into 48 parity-separated blocks of 8x16 = 128
pixels.  For each block the TensorEngine computes the banded product
P[m, (r,u)] = sum_c in1[c, y_m, x_m] * in2[c, r, u] over the block's
displacement window, accumulating fp32 in PSUM.

Inputs are fp8e4m3 hi+lo pairs (hi = e4m3(x), lo = e4m3(x - hi)) and every
matmul is a DoubleRow fp8 matmul with K=256 (the contraction channel space
packed as 2 k-tiles of 128 partitions) at 0.5 cycles/row.  Per chunk three
DoubleRow matmuls compute the 3-term product
    ah*bh + al*bh + ah*bl  =  a*b - al*bl  (al*bl ~ (0.4%)^2, negligible)
so PE time is 1.5 matmul-columns per banded column - 19.4 us/core, well
under the DMA pipe.  The first 26 chunks (the y0=0 sweep + 2) drop the ah*bl
term: their drains then depend only on the hi tensors, which starts the
drain-engine chain before the first in2-lo slice lands and pulls the whole
drain->store tail in by ~1.5 us.  Those entries carry the in2-side e4m3
error (~2.7%) instead of ~0.1%; measured end-to-end rel err is 1.61e-2
(int8 output quantization ~0.96e-2 + the 2-term region), still 19% under
the 2e-2 gate.

With inputs at 2 bytes per channel-pixel (hi+lo) and int8 banded stores the
kernel is bound by the serialized ~360 B/ns DMA pipe: 6.29 MB of loads +
3.97 MB of stores = 28.5 us.  The schedule keeps that pipe dense end to
end: all 18 loads are issued first in consumption order (hi slices ahead of
lo so each chunk's first two matmuls can fire before the lo data arrives,
and the hi*lo matmul + drain trail the hi stream by LAG chunks so late lo
slices never head-block the in-order PE queue).  The PE trails the load
stream and finishes ~4.5 us before the pipe; drains trail the PE; the quad
stores queue up behind the loads on the DMA engines with their drain
dependencies satisfied, so the pipe runs ~84% dense and the kernel ends
~1.5 us after the last store transfer (sem prop + exit barrier).

PSUM bands drain to SBUF as *int8* with a fixed scale (127/64 covers ~4
sigma of the dot-product distribution).  Each (block, half-band) gets one
PSUM bank (ring of 8); drains alternate ACT / DVE.  The 4 same-geometry
blocks of each (y0, x0) quad pack into one [128, 4*ntot] int8 staging tile
so every store keeps >=1872 B contiguous runs (full DMA rate).  Dummy
matmuls on a zeroed SBUF tile keep the PE p-state warm through the load
lead-in.  The host scatters the valid banded entries into the final output
(fixed sparse permutation) and undoes the int8 scale.
"""

import numpy as np

C, H, W = 256, 64, 96
D = 21
PADV = 20
B = 8
N_CORES = 8
BY, BX = 8, 16
NBLK = 48
QSTRIDE = 512          # psum bank size in fp32 elements
ST_FREE = 4096         # int8 staging/out free size per quad (4 * max ntot = 3744)
SCALE = 127.0 / 64.0   # int8 quantization scale (exact in fp32)

N_WARMUP = 26   # dummy matmuls warming the PE p-state through the load lead-in
N_FILLER = 8    # early chunks given an interleaved dummy to absorb load jitter
LAG = 0         # chunks the hi*lo matmul + drain trail the hi stream by
import os as _os
N_2TERM = int(_os.environ.get('K2', '24'))  # early chunks computed 2-term: their
                # drains don't wait for the first in2-lo slices, starting the
                # drain-engine chain ~1.5 us earlier

# load schedule: (kind 1=in1-blocks / 2=in2-rows, part 0=hi / 1=lo, begin, end)
# hi slices lead their lo twins so each chunk's first two matmuls (ah*bh,
# al*bh) can fire before the lo rhs arrives.
LOADS = [
    (2, 0, 0, 18), (1, 0, 0, 4), (1, 1, 0, 4), (2, 1, 0, 18),
    (1, 0, 4, 12), (1, 1, 4, 12),
    (2, 0, 18, 36), (2, 1, 18, 36), (1, 0, 12, 24), (1, 1, 12, 24),  # gy0
    (2, 0, 36, 52), (2, 1, 36, 52), (1, 0, 24, 36), (1, 1, 24, 36),  # gy1
    (2, 0, 52, 64), (2, 1, 52, 64), (1, 0, 36, 48), (1, 1, 36, 48),  # gy2/3
]


def _block_geometry():
    blocks = []
    for y0 in (0, 16, 32, 48):
        # large (x0=32, nu=36) quads first within each y0-group
        for x0 in (32, 0, 64):
            for py in (0, 1):
                for px in (0, 1):
                    ys = [y0 + py + 2 * b for b in range(BY)]
                    xs = [x0 + px + 2 * a for a in range(BX)]
                    r_lo = ys[0] - PADV
                    while r_lo < 0:
                        r_lo += 2
                    r_hi = min(ys[-1] + PADV, H - 1)
                    rs = list(range(r_lo, r_hi + 1, 2))
                    u_lo = xs[0] - PADV
                    while u_lo < 0:
                        u_lo += 2
                    u_hi = min(xs[-1] + PADV, W - 1)
                    us = list(range(u_lo, u_hi + 1, 2))
                    nu = len(us)
                    nr = len(rs)
                    # split rows in half: two pipelined half-bands per block,
                    # each one PSUM bank (<= 512 cols) draining as one op
                    n0 = nr // 2
                    assert n0 * nu <= QSTRIDE and (nr - n0) * nu <= QSTRIDE
                    chunks = [(0, n0), (n0, nr - n0)]
                    blocks.append(dict(ys=ys, xs=xs, rs=rs, us=us, chunks=chunks))
    assert len(blocks) == NBLK
    return blocks


_BLOCKS = _block_geometry()
_GATHER = None
_PROGRAM = None

# per-block pixel coordinates: YM[blk, m], XM[blk, m] with m = b*BX + a
_YM = np.array([np.repeat(g["ys"], BX) for g in _BLOCKS])
_XM = np.array([np.tile(g["xs"], BY) for g in _BLOCKS])


def _quad_ntot(qi):
    g = _BLOCKS[4 * qi]
    return len(g["rs"]) * len(g["us"])


def _build_gather():
    """Flat indices such that O.flat[dst] = R.flat[src] for one core."""
    dst_list, src_list = [], []
    for blk, g in enumerate(_BLOCKS):
        ys = np.asarray(g["ys"])
        xs = np.asarray(g["xs"])
        rs = np.asarray(g["rs"])
        us = np.asarray(g["us"])
        nu = len(us)
        ntot = len(rs) * nu
        y_m = np.repeat(ys, BX)
        x_m = np.tile(xs, BY)
        nr = len(rs)
        m_idx = np.arange(128)[:, None, None]
        ir = np.arange(nr)[None, :, None]
        iu = np.arange(nu)[None, None, :]
        i = (rs[None, :, None] - y_m[:, None, None] + PADV) // 2
        j = (us[None, None, :] - x_m[:, None, None] + PADV) // 2
        valid = (i >= 0) & (i < D) & (j >= 0) & (j < D)
        d = i * D + j
        dst = (d * H + y_m[:, None, None]) * W + x_m[:, None, None]
        src = ((blk // 4) * 128 + m_idx) * ST_FREE + (blk % 4) * ntot + ir * nu + iu
        bcast = np.broadcast_arrays(dst, src, valid)
        dst_list.append(bcast[0][valid])
        src_list.append(bcast[1][valid])
    return np.concatenate(dst_list), np.concatenate(src_list)


def _gather_indices():
    global _GATHER
    if _GATHER is None:
        _GATHER = _build_gather()
    return _GATHER


def _build_program():
    from contextlib import ExitStack

    import concourse.bacc as bacc
    import concourse.mybir as mybir
    import concourse.tile as tile

    f8 = mybir.dt.float8e4
    out_dt = mybir.dt.int8
    DR = mybir.MatmulPerfMode.DoubleRow

    nc = bacc.Bacc("TRN2", target_bir_lowering=False, debug=False)
    # in1 pre-packed on the host (pixel blocks), channel c = t*128 + p:
    #   in1h[p, blk, t, m] = e4m3(in1[c, YM[blk,m], XM[blk,m]]), in1l = residual
    # in2 feature map, same channel packing:
    #   in2h[p, r, t, u] = e4m3(in2[c, r, u]), in2l = residual
    in1h_d = nc.dram_tensor("in1h", [128, NBLK, 2, 128], f8, kind="ExternalInput")
    in1l_d = nc.dram_tensor("in1l", [128, NBLK, 2, 128], f8, kind="ExternalInput")
    in2h_d = nc.dram_tensor("in2h", [128, H, 2, W], f8, kind="ExternalInput")
    in2l_d = nc.dram_tensor("in2l", [128, H, 2, W], f8, kind="ExternalInput")
    out_d = nc.dram_tensor(
        "out", [NBLK // 4, 128, ST_FREE], out_dt, kind="ExternalOutput"
    )

    with ExitStack() as ctx:
        tc = ctx.enter_context(tile.TileContext(nc))
        inp_pool = ctx.enter_context(tc.tile_pool(name="inp", bufs=1))
        psum_pool = ctx.enter_context(tc.tile_pool(name="psum", bufs=8, space="PSUM"))
        out_pool = ctx.enter_context(tc.tile_pool(name="outp", bufs=12))

        in1h_s = inp_pool.tile([128, NBLK, 2, 128], f8)
        in1l_s = inp_pool.tile([128, NBLK, 2, 128], f8)
        in2h_s = inp_pool.tile([128, H, 2, W], f8)
        in2l_s = inp_pool.tile([128, H, 2, W], f8)
        wz = inp_pool.tile([128, 128], f8)

        # Input loads on the sync (SP) HWDGE path, in consumption order; the
        # 13 stores queue up behind them on the DMA engines with their drain
        # deps long satisfied, so the pipe stays dense to the end.
        def l1(part, b0, b1):
            s, d = (in1h_s, in1h_d) if part == 0 else (in1l_s, in1l_d)
            nc.sync.dma_start(s[:, b0:b1, :, :], d[:, b0:b1, :, :])

        def l2(part, r0, r1):
            s, d = (in2h_s, in2h_d) if part == 0 else (in2l_s, in2l_d)
            nc.sync.dma_start(s[:, r0:r1, :, :], d[:, r0:r1, :, :])

        for kind, part, a0, a1 in LOADS:
            (l1 if kind == 1 else l2)(part, a0, a1)

        # PE p-state warmup source: zeroed fp8 tile (Pool engine: free
        # earliest, so dummies start ~0.4 us)
        nc.gpsimd.memset(wz[:, :], 0.0)

        # --- half-band pipeline ----------------------------------------
        # Each (block, half) is a PSUM bank-granular unit: three DoubleRow
        # K=256 matmuls (ah*bh start, al*bh, ah*bl stop), then one scaled
        # int8 drain, alternating ACT / DVE.  The hi*lo matmul + drain trail
        # the hi*hi/lo*hi stream by LAG chunks so late lo slices never
        # head-block the in-order PE queue.  Chunk order per y0-group: all
        # first-halves then all second-halves (gy3 per-block for a short
        # store tail), so every matmul's in2 row window is resident when
        # the pipeline reaches it.
        chunk_list = []
        for gy in range(4):
            if gy < 3:
                for ci in (0, 1):
                    for blk in range(12 * gy, 12 * gy + 12):
                        chunk_list.append((blk, ci))
            else:
                for blk in range(36, 48):
                    for ci in (0, 1):
                        chunk_list.append((blk, ci))

        warm = psum_pool.tile([128, QSTRIDE], mybir.dt.float32, tag="bk", name="warm")
        for _ in range(N_WARMUP):
            nc.tensor.matmul(
                warm[:, :128], wz[:, :128], wz[:, :128], start=True, stop=True
            )

        bank_tiles = {}
        st_tiles = {}
        drained = {qi: 0 for qi in range(NBLK // 4)}
        n_drains = 0

        def chunk_geom(blk, ci):
            g = _BLOCKS[blk]
            off, n = g["chunks"][ci]
            nu = len(g["us"])
            u0 = g["us"][0]
            r0 = g["rs"][off]
            rsl = slice(r0, r0 + 2 * n - 1, 2)
            usl = slice(u0, u0 + 2 * nu - 1, 2)
            return g, off, n, nu, rsl, usl

        def emit_mm(blk, ci, two_term):
            """The hi*hi and lo*hi DoubleRow matmuls - open the PSUM bank."""
            g, off, n, nu, rsl, usl = chunk_geom(blk, ci)
            bk = psum_pool.tile(
                [128, QSTRIDE], mybir.dt.float32, tag="bk", name=f"bk{blk}_{ci}"
            )
            bank_tiles[(blk, ci)] = bk
            dst = bk[:, : n * nu]
            rhs_h = in2h_s[:, rsl, :, usl].transpose([0, 2, 1, 3])
            nc.tensor.matmul(
                dst, in1h_s[:, blk, :, :], rhs_h, start=True, stop=False,
                perf_mode=DR,
            )
            nc.tensor.matmul(
                dst, in1l_s[:, blk, :, :], rhs_h, start=False, stop=two_term,
                perf_mode=DR,
            )

        def emit_last_drain(blk, ci, two_term):
            """The hi*lo matmul (3-term chunks), then drain + quad stores."""
            nonlocal n_drains
            g, off, n, nu, rsl, usl = chunk_geom(blk, ci)
            ntot = len(g["rs"]) * nu
            bk = bank_tiles.pop((blk, ci))
            if not two_term:
                nc.tensor.matmul(
                    bk[:, : n * nu],
                    in1h_s[:, blk, :, :],
                    in2l_s[:, rsl, :, usl].transpose([0, 2, 1, 3]),
                    start=False,
                    stop=True,
                    perf_mode=DR,
                )
            qi = blk // 4
            if qi not in st_tiles:
                st_tiles[qi] = out_pool.tile(
                    [128, ST_FREE], out_dt, tag="st", name=f"st{qi}"
                )
            st = st_tiles[qi]
            base = (blk % 4) * ntot
            dst = st[:, base + off * nu : base + (off + n) * nu]
            if n_drains % 2 == 0:
                nc.scalar.mul(dst, bk[:, : n * nu], SCALE)
            else:
                nc.vector.tensor_scalar_mul(dst, bk[:, : n * nu], SCALE)
            n_drains += 1
            drained[qi] += 1
            width = 4 * ntot
            if qi == NBLK // 4 - 1:
                if drained[qi] == 4:
                    nc.sync.dma_start(
                        out_d[qi, :, : width // 2], st[:, : width // 2]
                    )
                elif drained[qi] == 8:
                    nc.sync.dma_start(
                        out_d[qi, :, width // 2 : width], st[:, width // 2 : width]
                    )
            elif drained[qi] == 8:
                nc.sync.dma_start(out_d[qi, :, :width], st[:, :width])

        pending = []
        for j, (blk, ci) in enumerate(chunk_list):
            emit_mm(blk, ci, j < N_2TERM)
            if j < N_FILLER:
                nc.tensor.matmul(
                    warm[:, :128], wz[:, :128], wz[:, :128], start=True, stop=True
                )
            pending.append((blk, ci, j < N_2TERM))
            while len(pending) > LAG:
                emit_last_drain(*pending.pop(0))
        while pending:
            emit_last_drain(*pending.pop(0))

    nc.compile()
    return nc


def _program():
    global _PROGRAM
    if _PROGRAM is None:
        _PROGRAM = _build_program()
    return _PROGRAM


def _f8():
    import ml_dtypes

    return ml_dtypes.float8_e4m3


def _hi_lo(x):
    e4 = _f8()
    hi = x.astype(e4)
    lo = (x - hi.astype(np.float32)).astype(e4)
    return hi, lo


def _prep_in1(x):
    """[256, 64, 96] -> (in1h, in1l) each [128, NBLK, 2, 128] e4m3.

    dim2 is the channel k-tile: element [p, blk, t, m] holds channel
    t*128 + p of pixel (YM[blk,m], XM[blk,m])."""
    g = x[:, _YM, _XM]                      # [256, NBLK, 128]
    g = g.reshape(2, 128, NBLK, 128)        # [t, p, blk, m]
    g = g.transpose(1, 2, 0, 3)             # [p, blk, t, m]
    hi, lo = _hi_lo(np.ascontiguousarray(g))
    return hi, lo


def _prep_in2(x):
    """[256, 64, 96] -> (in2h, in2l) each [128, H, 2, W] e4m3."""
    g = x.reshape(2, 128, H, W).transpose(1, 2, 0, 3)  # [p, r, t, u]
    hi, lo = _hi_lo(np.ascontiguousarray(g))
    return hi, lo


def make_in_maps(input1, input2):
    in1 = np.asarray(input1, dtype=np.float32)
    in2 = np.asarray(input2, dtype=np.float32)
    maps = []
    for b in range(B):
        a_h, a_l = _prep_in1(in1[b])
        b_h, b_l = _prep_in2(in2[b])
        maps.append({"in1h": a_h, "in1l": a_l, "in2h": b_h, "in2l": b_l})
    return maps


def extract_output(R):
    """R: [NBLK//4, 128, ST_FREE] int8 device result -> [441, 64, 96] fp32."""
    dst, src = _gather_indices()
    O = np.zeros(D * D * H * W, dtype=np.float32)
    O[dst] = R.reshape(-1)[src].astype(np.float32)
    O *= np.float32(1.0 / (SCALE * C))
    return O.reshape(D * D, H, W)


def run_spmd(in_maps, **kwargs):
    from concourse import bass_utils

    return bass_utils.run_bass_kernel_spmd(
        _program(), in_maps, core_ids=list(range(N_CORES)), **kwargs
    )


def kernel(input1, input2):
    in_maps = make_in_maps(input1, input2)
    res = run_spmd(in_maps)
    return np.stack([extract_output(res.results[b]["out"]) for b in range(B)])


# revision 47
# speedup vs baseline: 1.4045x; 1.0353x over previous
"""FlowNetC correlation (kernel_size=1, max_disp=20, stride2=2) on 8 Trainium2 cores.

Problem: inputs input1, input2 of shape [8, 256, 64, 96] fp32; output
[8, 441, 64, 96] fp32 with
  out[b, i*21+j, y, x] = (1/256) * sum_c in1[b,c,y,x] * in2[b,c,y+2i-20,x+2j-20]
(zero where the in2 index is out of range).

Sharding: data-parallel over batch - core b handles batch element b.

Per-core strategy: tile (y, x) into 48 parity-separated blocks of 8x16 = 128
pixels.  For each block the TensorEngine computes the banded product
P[m, (r,u)] = sum_c in1[c, y_m, x_m] * in2[c, r, u] over the block's
displacement window, accumulating fp32 in PSUM.

Inputs are fp8e4m3 hi+lo pairs (hi = e4m3(x), lo = e4m3(x - hi)) and every
matmul is a DoubleRow fp8 matmul with K=256 (the contraction channel space
packed as 2 k-tiles of 128 partitions) at 0.5 cycles/row.  Per chunk three
DoubleRow matmuls compute the 3-term product
    ah*bh + al*bh + ah*bl  =  a*b - al*bl  (al*bl ~ (0.4%)^2, negligible)
so PE time is 1.5 matmul-columns per banded column - 19.4 us/core, well
under the DMA pipe.  The first 26 chunks (the y0=0 sweep + 2) drop the ah*bl
term: their drains then depend only on the hi tensors, which starts the
drain-engine chain before the first in2-lo slice lands and pulls the whole
drain->store tail in by ~1.5 us.  Those entries carry the in2-side e4m3
error (~2.7%) instead of ~0.1%; measured end-to-end rel err is 1.61e-2
(int8 output quantization ~0.96e-2 + the 2-term region), still 19% under
the 2e-2 gate.

With inputs at 2 bytes per channel-pixel (hi+lo) and int8 banded stores the
kernel is bound by the serialized ~360 B/ns DMA pipe: 6.29 MB of loads +
3.97 MB of stores = 28.5 us.  The schedule keeps that pipe dense end to
end: all 18 loads are issued first in consumption order (hi slices ahead of
lo so each chunk's first two matmuls can fire before the lo data arrives,
and the hi*lo matmul + drain trail the hi stream by LAG chunks so late lo
slices never head-block the in-order PE queue).  The PE trails the load
stream and finishes ~4.5 us before the pipe; drains trail the PE; the quad
stores queue up behind the loads on the DMA engines with their drain
dependencies satisfied, so the pipe runs ~84% dense and the kernel ends
~1.5 us after the last store transfer (sem prop + exit barrier).

PSUM bands drain to SBUF as *int8* with a fixed scale (127/64 covers ~4
sigma of the dot-product distribution).  Each (block, half-band) gets one
PSUM bank (ring of 8); drains alternate ACT / DVE.  The 4 same-geometry
blocks of each (y0, x0) quad pack into one [128, 4*ntot] int8 staging tile
so every store keeps >=1872 B contiguous runs (full DMA rate).  Dummy
matmuls on a zeroed SBUF tile keep the PE p-state warm through the load
lead-in.  The host scatters the valid banded entries into the final output
(fixed sparse permutation) and undoes the int8 scale.
"""

import numpy as np

C, H, W = 256, 64, 96
D = 21
PADV = 20
B = 8
N_CORES = 8
BY, BX = 8, 16
NBLK = 48
QSTRIDE = 512          # psum bank size in fp32 elements
ST_FREE = 4096         # int8 staging/out free size per quad (4 * max ntot = 3744)
SCALE = 127.0 / 64.0   # int8 quantization scale (exact in fp32)

N_WARMUP = 26   # dummy matmuls warming the PE p-state through the load lead-in
N_FILLER = 8    # early chunks given an interleaved dummy to absorb load jitter
LAG = 0         # chunks the hi*lo matmul + drain trail the hi stream by
import os as _os
N_2TERM = int(_os.environ.get('K2', '24'))  # early chunks computed 2-term: their
                # drains don't wait for the first in2-lo slices, starting the
                # drain-engine chain ~1.5 us earlier

# load schedule: (kind 1=in1-blocks / 2=in2-rows, part 0=hi / 1=lo, begin, end)
# hi slices lead their lo twins so each chunk's first two matmuls (ah*bh,
# al*bh) can fire before the lo rhs arrives.
# in1l[0:12] is never loaded: blocks 0-11 run 2-term (ah*bh + ah*bl), so
# their in1-lo is unused and 0.39 MB drops off the serialized pipe.
LOADS = [
    (2, 0, 0, 18), (1, 0, 0, 4), (2, 1, 0, 18), (1, 0, 4, 12),
    (2, 0, 18, 36), (2, 1, 18, 36), (1, 0, 12, 24), (1, 1, 12, 24),  # gy0
    (2, 0, 36, 52), (2, 1, 36, 52), (1, 0, 24, 36), (1, 1, 24, 36),  # gy1
    (2, 0, 52, 64), (2, 1, 52, 64), (1, 0, 36, 48), (1, 1, 36, 48),  # gy2/3
]


def _block_geometry():
    blocks = []
    for y0 in (0, 16, 32, 48):
        # large (x0=32, nu=36) quads first within each y0-group
        for x0 in (32, 0, 64):
            for py in (0, 1):
                for px in (0, 1):
                    ys = [y0 + py + 2 * b for b in range(BY)]
                    xs = [x0 + px + 2 * a for a in range(BX)]
                    r_lo = ys[0] - PADV
                    while r_lo < 0:
                        r_lo += 2
                    r_hi = min(ys[-1] + PADV, H - 1)
                    rs = list(range(r_lo, r_hi + 1, 2))
                    u_lo = xs[0] - PADV
                    while u_lo < 0:
                        u_lo += 2
                    u_hi = min(xs[-1] + PADV, W - 1)
                    us = list(range(u_lo, u_hi + 1, 2))
                    nu = len(us)
                    nr = len(rs)
                    # split rows in half: two pipelined half-bands per block,
                    # each one PSUM bank (<= 512 cols) draining as one op
                    n0 = nr // 2
                    assert n0 * nu <= QSTRIDE and (nr - n0) * nu <= QSTRIDE
                    chunks = [(0, n0), (n0, nr - n0)]
                    blocks.append(dict(ys=ys, xs=xs, rs=rs, us=us, chunks=chunks))
    assert len(blocks) == NBLK
    return blocks


_BLOCKS = _block_geometry()
_GATHER = None
_PROGRAM = None

# per-block pixel coordinates: YM[blk, m], XM[blk, m] with m = b*BX + a
_YM = np.array([np.repeat(g["ys"], BX) for g in _BLOCKS])
_XM = np.array([np.tile(g["xs"], BY) for g in _BLOCKS])


def _quad_ntot(qi):
    g = _BLOCKS[4 * qi]
    return len(g["rs"]) * len(g["us"])


def _build_gather():
    """Flat indices such that O.flat[dst] = R.flat[src] for one core."""
    dst_list, src_list = [], []
    for blk, g in enumerate(_BLOCKS):
        ys = np.asarray(g["ys"])
        xs = np.asarray(g["xs"])
        rs = np.asarray(g["rs"])
        us = np.asarray(g["us"])
        nu = len(us)
        ntot = len(rs) * nu
        y_m = np.repeat(ys, BX)
        x_m = np.tile(xs, BY)
        nr = len(rs)
        m_idx = np.arange(128)[:, None, None]
        ir = np.arange(nr)[None, :, None]
        iu = np.arange(nu)[None, None, :]
        i = (rs[None, :, None] - y_m[:, None, None] + PADV) // 2
        j = (us[None, None, :] - x_m[:, None, None] + PADV) // 2
        valid = (i >= 0) & (i < D) & (j >= 0) & (j < D)
        d = i * D + j
        dst = (d * H + y_m[:, None, None]) * W + x_m[:, None, None]
        src = ((blk // 4) * 128 + m_idx) * ST_FREE + (blk % 4) * ntot + ir * nu + iu
        bcast = np.broadcast_arrays(dst, src, valid)
        dst_list.append(bcast[0][valid])
        src_list.append(bcast[1][valid])
    return np.concatenate(dst_list), np.concatenate(src_list)


def _gather_indices():
    global _GATHER
    if _GATHER is None:
        _GATHER = _build_gather()
    return _GATHER


def _build_program():
    from contextlib import ExitStack

    import concourse.bacc as bacc
    import concourse.mybir as mybir
    import concourse.tile as tile

    f8 = mybir.dt.float8e4
    out_dt = mybir.dt.int8
    DR = mybir.MatmulPerfMode.DoubleRow

    nc = bacc.Bacc("TRN2", target_bir_lowering=False, debug=False)
    # in1 pre-packed on the host (pixel blocks), channel c = t*128 + p:
    #   in1h[p, blk, t, m] = e4m3(in1[c, YM[blk,m], XM[blk,m]]), in1l = residual
    # in2 feature map, same channel packing:
    #   in2h[p, r, t, u] = e4m3(in2[c, r, u]), in2l = residual
    in1h_d = nc.dram_tensor("in1h", [128, NBLK, 2, 128], f8, kind="ExternalInput")
    in1l_d = nc.dram_tensor("in1l", [128, NBLK, 2, 128], f8, kind="ExternalInput")
    in2h_d = nc.dram_tensor("in2h", [128, H, 2, W], f8, kind="ExternalInput")
    in2l_d = nc.dram_tensor("in2l", [128, H, 2, W], f8, kind="ExternalInput")
    out_d = nc.dram_tensor(
        "out", [NBLK // 4, 128, ST_FREE], out_dt, kind="ExternalOutput"
    )

    with ExitStack() as ctx:
        tc = ctx.enter_context(tile.TileContext(nc))
        inp_pool = ctx.enter_context(tc.tile_pool(name="inp", bufs=1))
        psum_pool = ctx.enter_context(tc.tile_pool(name="psum", bufs=8, space="PSUM"))
        out_pool = ctx.enter_context(tc.tile_pool(name="outp", bufs=12))

        in1h_s = inp_pool.tile([128, NBLK, 2, 128], f8)
        in1l_s = inp_pool.tile([128, NBLK, 2, 128], f8)
        in2h_s = inp_pool.tile([128, H, 2, W], f8)
        in2l_s = inp_pool.tile([128, H, 2, W], f8)
        wz = inp_pool.tile([128, 128], f8)

        # Input loads on the sync (SP) HWDGE path, in consumption order; the
        # 13 stores queue up behind them on the DMA engines with their drain
        # deps long satisfied, so the pipe stays dense to the end.
        def l1(part, b0, b1):
            s, d = (in1h_s, in1h_d) if part == 0 else (in1l_s, in1l_d)
            nc.sync.dma_start(s[:, b0:b1, :, :], d[:, b0:b1, :, :])

        def l2(part, r0, r1):
            s, d = (in2h_s, in2h_d) if part == 0 else (in2l_s, in2l_d)
            nc.sync.dma_start(s[:, r0:r1, :, :], d[:, r0:r1, :, :])

        for kind, part, a0, a1 in LOADS:
            (l1 if kind == 1 else l2)(part, a0, a1)

        # PE p-state warmup source: zeroed fp8 tile (Pool engine: free
        # earliest, so dummies start ~0.4 us)
        nc.gpsimd.memset(wz[:, :], 0.0)

        # --- half-band pipeline ----------------------------------------
        # Each (block, half) is a PSUM bank-granular unit: three DoubleRow
        # K=256 matmuls (ah*bh start, al*bh, ah*bl stop), then one scaled
        # int8 drain, alternating ACT / DVE.  The hi*lo matmul + drain trail
        # the hi*hi/lo*hi stream by LAG chunks so late lo slices never
        # head-block the in-order PE queue.  Chunk order per y0-group: all
        # first-halves then all second-halves (gy3 per-block for a short
        # store tail), so every matmul's in2 row window is resident when
        # the pipeline reaches it.
        # (blk, ci, merged): merged blocks put both half-bands in ONE PSUM
        # bank (half <= 256 cols) and drain them as a single op - gy0/gy3's
        # x0 in {0,64} blocks qualify; their halves are 234 cols.  Those are
        # emitted per-block between the big x32 sweeps so the shared bank
        # only lives two chunk slots.
        chunk_list = []
        for gy in range(3):
            for ci in (0, 1):
                for blk in range(12 * gy, 12 * gy + 12):
                    chunk_list.append((blk, ci, False))
        for blk in range(36, 48):
            merged = blk >= 40  # x0 in {0, 64}: halves are 234 cols
            chunk_list.append((blk, 0, merged))
            chunk_list.append((blk, 1, merged))

        warm = psum_pool.tile([128, QSTRIDE], mybir.dt.float32, tag="bk", name="warm")
        for _ in range(N_WARMUP):
            nc.tensor.matmul(
                warm[:, :128], wz[:, :128], wz[:, :128], start=True, stop=True
            )

        bank_tiles = {}
        st_tiles = {}
        drained = {qi: 0 for qi in range(NBLK // 4)}
        n_drains = 0

        def chunk_geom(blk, ci):
            g = _BLOCKS[blk]
            off, n = g["chunks"][ci]
            nu = len(g["us"])
            u0 = g["us"][0]
            r0 = g["rs"][off]
            rsl = slice(r0, r0 + 2 * n - 1, 2)
            usl = slice(u0, u0 + 2 * nu - 1, 2)
            return g, off, n, nu, rsl, usl

        def emit_mm(blk, ci, merged, two_term):
            """The hi*hi (and, for 3-term chunks, lo*hi) DoubleRow matmuls
            - open the PSUM bank."""
            g, off, n, nu, rsl, usl = chunk_geom(blk, ci)
            if merged and ci == 1:
                bk = bank_tiles[(blk, 0)]
            else:
                bk = psum_pool.tile(
                    [128, QSTRIDE], mybir.dt.float32, tag="bk",
                    name=f"bk{blk}_{ci}"
                )
            bank_tiles[(blk, ci)] = bk
            half = g["chunks"][0][1] * nu
            base = half * ci if merged else 0
            dst = bk[:, base : base + n * nu]
            rhs_h = in2h_s[:, rsl, :, usl].transpose([0, 2, 1, 3])
            nc.tensor.matmul(
                dst, in1h_s[:, blk, :, :], rhs_h, start=True, stop=False,
                perf_mode=DR,
            )
            if not two_term:
                nc.tensor.matmul(
                    dst, in1l_s[:, blk, :, :], rhs_h, start=False, stop=False,
                    perf_mode=DR,
                )

        def emit_last_drain(blk, ci, merged, two_term):
            """The hi*lo matmul, then the int8 drain and quad stores."""
            nonlocal n_drains
            g, off, n, nu, rsl, usl = chunk_geom(blk, ci)
            ntot = len(g["rs"]) * nu
            if merged and ci == 0:
                bk = bank_tiles[(blk, ci)]  # stays live until ci 1 drains
            else:
                bk = bank_tiles.pop((blk, ci))
                if merged:
                    bank_tiles.pop((blk, 0), None)
            half = g["chunks"][0][1] * nu
            pbase = half * ci if merged else 0
            nc.tensor.matmul(
                bk[:, pbase : pbase + n * nu],
                in1h_s[:, blk, :, :],
                in2l_s[:, rsl, :, usl].transpose([0, 2, 1, 3]),
                start=False,
                stop=True,
                perf_mode=DR,
            )
            if merged and ci == 0:
                return  # both halves drain together after ci 1
            qi = blk // 4
            if qi not in st_tiles:
                st_tiles[qi] = out_pool.tile(
                    [128, ST_FREE], out_dt, tag="st", name=f"st{qi}"
                )
            st = st_tiles[qi]
            base = (blk % 4) * ntot
            if merged:
                src_ap = bk[:, :ntot]
                dst = st[:, base : base + ntot]
            else:
                src_ap = bk[:, : n * nu]
                dst = st[:, base + off * nu : base + (off + n) * nu]
            if n_drains % 2 == 0:
                nc.scalar.mul(dst, src_ap, SCALE)
            else:
                nc.vector.tensor_scalar_mul(dst, src_ap, SCALE)
            n_drains += 1
            drained[qi] += 2 if merged else 1
            width = 4 * ntot
            if qi == NBLK // 4 - 1:
                if drained[qi] == 4:
                    nc.sync.dma_start(
                        out_d[qi, :, : width // 2], st[:, : width // 2]
                    )
                elif drained[qi] == 8:
                    nc.sync.dma_start(
                        out_d[qi, :, width // 2 : width], st[:, width // 2 : width]
                    )
            elif drained[qi] == 8:
                nc.sync.dma_start(out_d[qi, :, :width], st[:, :width])

        pending = []
        for j, (blk, ci, merged) in enumerate(chunk_list):
            two_term = blk < 12 or (blk < 14 and ci == 0)
            emit_mm(blk, ci, merged, two_term)
            if j < N_FILLER:
                nc.tensor.matmul(
                    warm[:, :128], wz[:, :128], wz[:, :128], start=True, stop=True
                )
            pending.append((blk, ci, merged, two_term))
            while len(pending) > LAG:
                emit_last_drain(*pending.pop(0))
        while pending:
            emit_last_drain(*pending.pop(0))

    nc.compile()
    return nc


def _program():
    global _PROGRAM
    if _PROGRAM is None:
        _PROGRAM = _build_program()
    return _PROGRAM


def _f8():
    import ml_dtypes

    return ml_dtypes.float8_e4m3


def _hi_lo(x):
    e4 = _f8()
    hi = x.astype(e4)
    lo = (x - hi.astype(np.float32)).astype(e4)
    return hi, lo


def _prep_in1(x):
    """[256, 64, 96] -> (in1h, in1l) each [128, NBLK, 2, 128] e4m3.

    dim2 is the channel k-tile: element [p, blk, t, m] holds channel
    t*128 + p of pixel (YM[blk,m], XM[blk,m])."""
    g = x[:, _YM, _XM]                      # [256, NBLK, 128]
    g = g.reshape(2, 128, NBLK, 128)        # [t, p, blk, m]
    g = g.transpose(1, 2, 0, 3)             # [p, blk, t, m]
    hi, lo = _hi_lo(np.ascontiguousarray(g))
    return hi, lo


def _prep_in2(x):
    """[256, 64, 96] -> (in2h, in2l) each [128, H, 2, W] e4m3."""
    g = x.reshape(2, 128, H, W).transpose(1, 2, 0, 3)  # [p, r, t, u]
    hi, lo = _hi_lo(np.ascontiguousarray(g))
    return hi, lo


def make_in_maps(input1, input2):
    in1 = np.asarray(input1, dtype=np.float32)
    in2 = np.asarray(input2, dtype=np.float32)
    maps = []
    for b in range(B):
        a_h, a_l = _prep_in1(in1[b])
        b_h, b_l = _prep_in2(in2[b])
        maps.append({"in1h": a_h, "in1l": a_l, "in2h": b_h, "in2l": b_l})
    return maps


def extract_output(R):
    """R: [NBLK//4, 128, ST_FREE] int8 device result -> [441, 64, 96] fp32."""
    dst, src = _gather_indices()
    O = np.zeros(D * D * H * W, dtype=np.float32)
    O[dst] = R.reshape(-1)[src].astype(np.float32)
    O *= np.float32(1.0 / (SCALE * C))
    return O.reshape(D * D, H, W)


def run_spmd(in_maps, **kwargs):
    from concourse import bass_utils

    return bass_utils.run_bass_kernel_spmd(
        _program(), in_maps, core_ids=list(range(N_CORES)), **kwargs
    )


def kernel(input1, input2):
    in_maps = make_in_maps(input1, input2)
    res = run_spmd(in_maps)
    return np.stack([extract_output(res.results[b]["out"]) for b in range(B)])


# revision 53
# speedup vs baseline: 1.4097x; 1.0037x over previous
"""FlowNetC correlation (kernel_size=1, max_disp=20, stride2=2) on 8 Trainium2 cores.

Problem: inputs input1, input2 of shape [8, 256, 64, 96] fp32; output
[8, 441, 64, 96] fp32 with
  out[b, i*21+j, y, x] = (1/256) * sum_c in1[b,c,y,x] * in2[b,c,y+2i-20,x+2j-20]
(zero where the in2 index is out of range).

Sharding: data-parallel over batch - core b handles batch element b.

Per-core strategy: tile (y, x) into 48 parity-separated blocks of 8x16 = 128
pixels.  For each block the TensorEngine computes the banded product
P[m, (r,u)] = sum_c in1[c, y_m, x_m] * in2[c, r, u] over the block's
displacement window, accumulating fp32 in PSUM.

Inputs are fp8e4m3 hi+lo pairs (hi = e4m3(x), lo = e4m3(x - hi)) and every
matmul is a DoubleRow fp8 matmul with K=256 (the contraction channel space
packed as 2 k-tiles of 128 partitions) at 0.5 cycles/row.  Per chunk three
DoubleRow matmuls compute the 3-term product
    ah*bh + al*bh + ah*bl  =  a*b - al*bl  (al*bl ~ (0.4%)^2, negligible)
so PE time is 1.5 matmul-columns per banded column - 19.4 us/core, well
under the DMA pipe.  The first 26 chunks (the y0=0 sweep + 2) drop the al*bh
term: blocks 0-11 never read in1-lo (its first slices are never even
loaded - 0.39 MB off the pipe) and the drain-engine chain starts early,
pulling the whole
drain->store tail in.  Those entries carry the in1-side e4m3 error (~2.7%)
instead of ~0.1%; measured end-to-end rel err is 1.61e-2 (int8 output
quantization ~0.96e-2 + the 2-term region), still 19% under the 2e-2 gate.

With inputs at 2 bytes per channel-pixel (hi+lo) and int8 banded stores the
kernel is bound by the serialized ~360 B/ns DMA pipe: 5.90 MB of loads +
3.97 MB of stores = 27.4 us.  The schedule keeps that pipe dense end to
end: all 18 loads are issued first in consumption order (hi slices ahead of
lo so each chunk's first two matmuls can fire before the lo data arrives,
and the hi*lo matmul + drain trail the hi stream by LAG chunks so late lo
slices never head-block the in-order PE queue).  The PE trails the load
stream and finishes ~4.5 us before the pipe; drains trail the PE; the quad
stores queue up behind the loads on the DMA engines with their drain
dependencies satisfied, so the pipe runs ~84% dense and the kernel ends
~1.5 us after the last store transfer (sem prop + exit barrier).

PSUM bands drain to SBUF as *int8* with a fixed scale (127/64 covers ~4
sigma of the dot-product distribution).  Each (block, half-band) gets one
PSUM bank (ring of 8); drains alternate ACT / DVE.  In the final y0-group
the small (x0 in {0, 64}) blocks pack both 234-col half-bands into ONE bank
and drain them as a single op, trimming the drain chain right where it
gates the last stores.  The 4 same-geometry
blocks of each (y0, x0) quad pack into one [128, 4*ntot] int8 staging tile
so every store keeps >=1872 B contiguous runs (full DMA rate).  Dummy
matmuls on a zeroed SBUF tile keep the PE p-state warm through the load
lead-in.  The host scatters the valid banded entries into the final output
(fixed sparse permutation) and undoes the int8 scale.
"""

import numpy as np

C, H, W = 256, 64, 96
D = 21
PADV = 20
B = 8
N_CORES = 8
BY, BX = 8, 16
NBLK = 48
QSTRIDE = 512          # psum bank size in fp32 elements
ST_FREE = 4096         # int8 staging/out free size per quad (4 * max ntot = 3744)
SCALE = 127.0 / 64.0   # int8 quantization scale (exact in fp32)

N_WARMUP = 26   # dummy matmuls warming the PE p-state through the load lead-in
N_FILLER = 8    # early chunks given an interleaved dummy to absorb load jitter
LAG = 0         # chunks the hi*lo matmul + drain trail the hi stream by
import os as _os
N_2TERM = int(_os.environ.get('K2', '24'))  # early chunks computed 2-term: their
                # drains don't wait for the first in2-lo slices, starting the
                # drain-engine chain ~1.5 us earlier

# load schedule: (kind 1=in1-blocks / 2=in2-rows, part 0=hi / 1=lo, begin, end)
# hi slices lead their lo twins so each chunk's first two matmuls (ah*bh,
# al*bh) can fire before the lo rhs arrives.
# in1l[0:12] is never loaded: blocks 0-11 run 2-term (ah*bh + ah*bl), so
# their in1-lo is unused and 0.39 MB drops off the serialized pipe.
LOADS = [
    (2, 0, 0, 18), (1, 0, 0, 4), (2, 1, 0, 18), (1, 0, 4, 12),
    (2, 0, 18, 36), (2, 1, 18, 36), (1, 0, 12, 24), (1, 1, 12, 24),  # gy0
    (2, 0, 36, 52), (2, 1, 36, 52), (1, 0, 24, 36), (1, 1, 24, 36),  # gy1
    (2, 0, 52, 64), (2, 1, 52, 64), (1, 0, 36, 48), (1, 1, 36, 48),  # gy2/3
]


def _block_geometry():
    blocks = []
    for y0 in (0, 16, 32, 48):
        # large (x0=32, nu=36) quads first within each y0-group
        for x0 in (32, 0, 64):
            for py in (0, 1):
                for px in (0, 1):
                    ys = [y0 + py + 2 * b for b in range(BY)]
                    xs = [x0 + px + 2 * a for a in range(BX)]
                    r_lo = ys[0] - PADV
                    while r_lo < 0:
                        r_lo += 2
                    r_hi = min(ys[-1] + PADV, H - 1)
                    rs = list(range(r_lo, r_hi + 1, 2))
                    u_lo = xs[0] - PADV
                    while u_lo < 0:
                        u_lo += 2
                    u_hi = min(xs[-1] + PADV, W - 1)
                    us = list(range(u_lo, u_hi + 1, 2))
                    nu = len(us)
                    nr = len(rs)
                    # split rows in half: two pipelined half-bands per block,
                    # each one PSUM bank (<= 512 cols) draining as one op
                    n0 = nr // 2
                    assert n0 * nu <= QSTRIDE and (nr - n0) * nu <= QSTRIDE
                    chunks = [(0, n0), (n0, nr - n0)]
                    blocks.append(dict(ys=ys, xs=xs, rs=rs, us=us, chunks=chunks))
    assert len(blocks) == NBLK
    return blocks


_BLOCKS = _block_geometry()
_GATHER = None
_PROGRAM = None

# per-block pixel coordinates: YM[blk, m], XM[blk, m] with m = b*BX + a
_YM = np.array([np.repeat(g["ys"], BX) for g in _BLOCKS])
_XM = np.array([np.tile(g["xs"], BY) for g in _BLOCKS])


def _quad_ntot(qi):
    g = _BLOCKS[4 * qi]
    return len(g["rs"]) * len(g["us"])


def _build_gather():
    """Flat indices such that O.flat[dst] = R.flat[src] for one core."""
    dst_list, src_list = [], []
    for blk, g in enumerate(_BLOCKS):
        ys = np.asarray(g["ys"])
        xs = np.asarray(g["xs"])
        rs = np.asarray(g["rs"])
        us = np.asarray(g["us"])
        nu = len(us)
        ntot = len(rs) * nu
        y_m = np.repeat(ys, BX)
        x_m = np.tile(xs, BY)
        nr = len(rs)
        m_idx = np.arange(128)[:, None, None]
        ir = np.arange(nr)[None, :, None]
        iu = np.arange(nu)[None, None, :]
        i = (rs[None, :, None] - y_m[:, None, None] + PADV) // 2
        j = (us[None, None, :] - x_m[:, None, None] + PADV) // 2
        valid = (i >= 0) & (i < D) & (j >= 0) & (j < D)
        d = i * D + j
        dst = (d * H + y_m[:, None, None]) * W + x_m[:, None, None]
        src = ((blk // 4) * 128 + m_idx) * ST_FREE + (blk % 4) * ntot + ir * nu + iu
        bcast = np.broadcast_arrays(dst, src, valid)
        dst_list.append(bcast[0][valid])
        src_list.append(bcast[1][valid])
    return np.concatenate(dst_list), np.concatenate(src_list)


def _gather_indices():
    global _GATHER
    if _GATHER is None:
        _GATHER = _build_gather()
    return _GATHER


def _build_program():
    from contextlib import ExitStack

    import concourse.bacc as bacc
    import concourse.mybir as mybir
    import concourse.tile as tile

    f8 = mybir.dt.float8e4
    out_dt = mybir.dt.int8
    DR = mybir.MatmulPerfMode.DoubleRow

    nc = bacc.Bacc("TRN2", target_bir_lowering=False, debug=False)
    # in1 pre-packed on the host (pixel blocks), channel c = t*128 + p:
    #   in1h[p, blk, t, m] = e4m3(in1[c, YM[blk,m], XM[blk,m]]), in1l = residual
    # in2 feature map, same channel packing:
    #   in2h[p, r, t, u] = e4m3(in2[c, r, u]), in2l = residual
    in1h_d = nc.dram_tensor("in1h", [128, NBLK, 2, 128], f8, kind="ExternalInput")
    in1l_d = nc.dram_tensor("in1l", [128, NBLK, 2, 128], f8, kind="ExternalInput")
    in2h_d = nc.dram_tensor("in2h", [128, H, 2, W], f8, kind="ExternalInput")
    in2l_d = nc.dram_tensor("in2l", [128, H, 2, W], f8, kind="ExternalInput")
    out_d = nc.dram_tensor(
        "out", [NBLK // 4, 128, ST_FREE], out_dt, kind="ExternalOutput"
    )

    with ExitStack() as ctx:
        tc = ctx.enter_context(tile.TileContext(nc))
        inp_pool = ctx.enter_context(tc.tile_pool(name="inp", bufs=1))
        psum_pool = ctx.enter_context(tc.tile_pool(name="psum", bufs=8, space="PSUM"))
        out_pool = ctx.enter_context(tc.tile_pool(name="outp", bufs=12))

        in1h_s = inp_pool.tile([128, NBLK, 2, 128], f8)
        in1l_s = inp_pool.tile([128, NBLK, 2, 128], f8)
        in2h_s = inp_pool.tile([128, H, 2, W], f8)
        in2l_s = inp_pool.tile([128, H, 2, W], f8)
        wz = inp_pool.tile([128, 128], f8)

        # Input loads on the sync (SP) HWDGE path, in consumption order; the
        # 13 stores queue up behind them on the DMA engines with their drain
        # deps long satisfied, so the pipe stays dense to the end.
        def l1(part, b0, b1):
            s, d = (in1h_s, in1h_d) if part == 0 else (in1l_s, in1l_d)
            nc.sync.dma_start(s[:, b0:b1, :, :], d[:, b0:b1, :, :])

        def l2(part, r0, r1):
            s, d = (in2h_s, in2h_d) if part == 0 else (in2l_s, in2l_d)
            nc.sync.dma_start(s[:, r0:r1, :, :], d[:, r0:r1, :, :])

        for kind, part, a0, a1 in LOADS:
            (l1 if kind == 1 else l2)(part, a0, a1)

        # PE p-state warmup source: zeroed fp8 tile (Pool engine: free
        # earliest, so dummies start ~0.4 us)
        nc.gpsimd.memset(wz[:, :], 0.0)

        # --- half-band pipeline ----------------------------------------
        # Each (block, half) is a PSUM bank-granular unit: three DoubleRow
        # K=256 matmuls (ah*bh start, al*bh, ah*bl stop), then one scaled
        # int8 drain, alternating ACT / DVE.  The hi*lo matmul + drain trail
        # the hi*hi/lo*hi stream by LAG chunks so late lo slices never
        # head-block the in-order PE queue.  Chunk order per y0-group: all
        # first-halves then all second-halves (gy3 per-block for a short
        # store tail), so every matmul's in2 row window is resident when
        # the pipeline reaches it.
        # (blk, ci, merged): merged blocks put both half-bands in ONE PSUM
        # bank (half <= 256 cols) and drain them as a single op - gy0/gy3's
        # x0 in {0,64} blocks qualify; their halves are 234 cols.  Those are
        # emitted per-block between the big x32 sweeps so the shared bank
        # only lives two chunk slots.
        chunk_list = []
        for gy in range(3):
            for ci in (0, 1):
                for blk in range(12 * gy, 12 * gy + 12):
                    chunk_list.append((blk, ci, False))
        for blk in range(36, 48):
            merged = blk >= 40  # x0 in {0, 64}: halves are 234 cols
            chunk_list.append((blk, 0, merged))
            chunk_list.append((blk, 1, merged))

        warm = psum_pool.tile([128, QSTRIDE], mybir.dt.float32, tag="bk", name="warm")
        for _ in range(N_WARMUP):
            nc.tensor.matmul(
                warm[:, :128], wz[:, :128], wz[:, :128], start=True, stop=True
            )

        bank_tiles = {}
        st_tiles = {}
        drained = {qi: 0 for qi in range(NBLK // 4)}
        n_drains = 0

        def chunk_geom(blk, ci):
            g = _BLOCKS[blk]
            off, n = g["chunks"][ci]
            nu = len(g["us"])
            u0 = g["us"][0]
            r0 = g["rs"][off]
            rsl = slice(r0, r0 + 2 * n - 1, 2)
            usl = slice(u0, u0 + 2 * nu - 1, 2)
            return g, off, n, nu, rsl, usl

        def emit_mm(blk, ci, merged, two_term):
            """The hi*hi (and, for 3-term chunks, lo*hi) DoubleRow matmuls
            - open the PSUM bank."""
            g, off, n, nu, rsl, usl = chunk_geom(blk, ci)
            if merged and ci == 1:
                bk = bank_tiles[(blk, 0)]
            else:
                bk = psum_pool.tile(
                    [128, QSTRIDE], mybir.dt.float32, tag="bk",
                    name=f"bk{blk}_{ci}"
                )
            bank_tiles[(blk, ci)] = bk
            half = g["chunks"][0][1] * nu
            base = half * ci if merged else 0
            dst = bk[:, base : base + n * nu]
            rhs_h = in2h_s[:, rsl, :, usl].transpose([0, 2, 1, 3])
            nc.tensor.matmul(
                dst, in1h_s[:, blk, :, :], rhs_h, start=True, stop=False,
                perf_mode=DR,
            )
            if not two_term:
                nc.tensor.matmul(
                    dst, in1l_s[:, blk, :, :], rhs_h, start=False, stop=False,
                    perf_mode=DR,
                )

        def emit_last_drain(blk, ci, merged, two_term):
            """The hi*lo matmul, then the int8 drain and quad stores."""
            nonlocal n_drains
            g, off, n, nu, rsl, usl = chunk_geom(blk, ci)
            ntot = len(g["rs"]) * nu
            if merged and ci == 0:
                bk = bank_tiles[(blk, ci)]  # stays live until ci 1 drains
            else:
                bk = bank_tiles.pop((blk, ci))
                if merged:
                    bank_tiles.pop((blk, 0), None)
            half = g["chunks"][0][1] * nu
            pbase = half * ci if merged else 0
            nc.tensor.matmul(
                bk[:, pbase : pbase + n * nu],
                in1h_s[:, blk, :, :],
                in2l_s[:, rsl, :, usl].transpose([0, 2, 1, 3]),
                start=False,
                stop=True,
                perf_mode=DR,
            )
            if merged and ci == 0:
                return  # both halves drain together after ci 1
            qi = blk // 4
            if qi not in st_tiles:
                st_tiles[qi] = out_pool.tile(
                    [128, ST_FREE], out_dt, tag="st", name=f"st{qi}"
                )
            st = st_tiles[qi]
            base = (blk % 4) * ntot
            if merged:
                src_ap = bk[:, :ntot]
                dst = st[:, base : base + ntot]
            else:
                src_ap = bk[:, : n * nu]
                dst = st[:, base + off * nu : base + (off + n) * nu]
            if n_drains % 2 == 1:
                nc.scalar.mul(dst, src_ap, SCALE)
            else:
                nc.vector.tensor_scalar_mul(dst, src_ap, SCALE)
            n_drains += 1
            drained[qi] += 2 if merged else 1
            # quads 9 and 11 store in halves: q11 so the final piece (and
            # its DGE+sem latency) is small and early, q9 so its first half
            # fills the pipe while gy3's later blocks still drain
            width = 4 * ntot
            if qi in (NBLK // 4 - 3, NBLK // 4 - 1):
                if drained[qi] == 4:
                    nc.sync.dma_start(
                        out_d[qi, :, : width // 2], st[:, : width // 2]
                    )
                elif drained[qi] == 8:
                    nc.sync.dma_start(
                        out_d[qi, :, width // 2 : width], st[:, width // 2 : width]
                    )
            elif drained[qi] == 8:
                nc.sync.dma_start(out_d[qi, :, :width], st[:, :width])

        pending = []
        for j, (blk, ci, merged) in enumerate(chunk_list):
            two_term = blk < 12 or (blk < 14 and ci == 0)  # 26 chunks
            emit_mm(blk, ci, merged, two_term)
            if j < N_FILLER:
                nc.tensor.matmul(
                    warm[:, :128], wz[:, :128], wz[:, :128], start=True, stop=True
                )
            pending.append((blk, ci, merged, two_term))
            while len(pending) > LAG:
                emit_last_drain(*pending.pop(0))
        while pending:
            emit_last_drain(*pending.pop(0))

    nc.compile()
    return nc


def _program():
    global _PROGRAM
    if _PROGRAM is None:
        _PROGRAM = _build_program()
    return _PROGRAM


def _f8():
    import ml_dtypes

    return ml_dtypes.float8_e4m3


def _hi_lo(x):
    e4 = _f8()
    hi = x.astype(e4)
    lo = (x - hi.astype(np.float32)).astype(e4)
    return hi, lo


def _prep_in1(x):
    """[256, 64, 96] -> (in1h, in1l) each [128, NBLK, 2, 128] e4m3.

    dim2 is the channel k-tile: element [p, blk, t, m] holds channel
    t*128 + p of pixel (YM[blk,m], XM[blk,m])."""
    g = x[:, _YM, _XM]                      # [256, NBLK, 128]
    g = g.reshape(2, 128, NBLK, 128)        # [t, p, blk, m]
    g = g.transpose(1, 2, 0, 3)             # [p, blk, t, m]
    hi, lo = _hi_lo(np.ascontiguousarray(g))
    return hi, lo


def _prep_in2(x):
    """[256, 64, 96] -> (in2h, in2l) each [128, H, 2, W] e4m3."""
    g = x.reshape(2, 128, H, W).transpose(1, 2, 0, 3)  # [p, r, t, u]
    hi, lo = _hi_lo(np.ascontiguousarray(g))
    return hi, lo


def make_in_maps(input1, input2):
    in1 = np.asarray(input1, dtype=np.float32)
    in2 = np.asarray(input2, dtype=np.float32)
    maps = []
    for b in range(B):
        a_h, a_l = _prep_in1(in1[b])
        b_h, b_l = _prep_in2(in2[b])
        maps.append({"in1h": a_h, "in1l": a_l, "in2h": b_h, "in2l": b_l})
    return maps


def extract_output(R):
    """R: [NBLK//4, 128, ST_FREE] int8 device result -> [441, 64, 96] fp32."""
    dst, src = _gather_indices()
    O = np.zeros(D * D * H * W, dtype=np.float32)
    O[dst] = R.reshape(-1)[src].astype(np.float32)
    O *= np.float32(1.0 / (SCALE * C))
    return O.reshape(D * D, H, W)


def run_spmd(in_maps, **kwargs):
    from concourse import bass_utils

    return bass_utils.run_bass_kernel_spmd(
        _program(), in_maps, core_ids=list(range(N_CORES)), **kwargs
    )


def kernel(input1, input2):
    in_maps = make_in_maps(input1, input2)
    res = run_spmd(in_maps)
    return np.stack([extract_output(res.results[b]["out"]) for b in range(B)])
